# revision 33
# baseline (speedup 1.0000x reference)
"""Differential attention kernel for Trainium2 (8 NeuronCores).

v6: doubly-reassociated algebra + cross-body software pipelining +
persistent cross-body streaming pools.

out = diff_attn @ V @ Wo is evaluated as p @ (x @ (Wv @ Wo)): both matrix
products are reassociated so the 8192-wide v dimension is contracted once
in the weight-weight product WVO = Wv @ Wo [512, 512], computed on-device
every iteration, cooperatively across all 8 cores (each contracts a
1024-wide v slice; a 1MB f32 AllReduce sums the partials). VW = x @ WVO
then costs S*D*DM instead of the S*VD*(D+DM) of the naive order.

Sharding: 2 batch groups x 4 cores; core g of a group handles q rows
[g*512, (g+1)*512). Per body:
  - WVO partial (v-slice) -> AllReduce across all 8 cores (weights are
    batch-independent, so one global reduction serves both groups),
  - qkv projection + scores/softmax/combine/transpose for the q quarter,
  - deferred tail (next body): VW = x @ WVO for the full batch, then
    p @ VW for the q quarter -> out tile [512, 512].
Host concatenates the 8 disjoint output tiles; bv/bo fold into a constant
host-side correction using sum_k(diff_attn[q,:]) == 1 - lam.

Two pipelining mechanisms:
 - kernel_body is a generator: body i's AllReduce-gated tail is emitted
   after body i+1's phase 1, so the in-order PE queue never stalls on the
   collective.
 - input stream, qkvT, and scratch live in pools created ONCE and shared
   by all bodies (tag-rotated tiles): body i+1's loads land during body
   i's compute. Only the small carry state (ptile/r1) and DRAM collective
   tiles are per-body, side-alternated for the allocator's LIFO stacks.
"""

import math
from contextlib import ExitStack

import numpy as np
import ml_dtypes

import concourse.bass as bass
from concourse import bacc
import concourse.mybir as mybir
import concourse.tile as tile
from concourse import bass_utils
from concourse.bass import ts, ds
from concourse.masks import make_identity

# Problem shapes (hardcoded per harness contract).
B = 2
S = 2048
D = 512
VDIM = 8192
DM = 512
P = 128
G = 4                 # cores per batch group
SQ = S // G           # 512 q rows per core
VS = VDIM // 8        # 1024 v per core for the WVO partial
SCALE = 1.0 / math.sqrt(64.0)
LAMBDA_INIT = 0.8
LAYER_INDEX = 0

F32 = mybir.dt.float32
F32R = mybir.dt.float32r
BF16 = mybir.dt.bfloat16
EXP = mybir.ActivationFunctionType.Exp
IDENT = mybir.ActivationFunctionType.Identity
AXX = mybir.AxisListType.X

KD = D // P           # 4 contraction chunks of the input dim
MQ = (2 * D) // P     # 8 m-chunks of qkv output dim
SN = S // 512         # 4 free chunks of S
NKC = S // P          # 16 k-chunks of 128
QB = SQ // P          # 4 q-blocks per core
VC8 = VS // P         # 8 v-chunks of 128 in this core's WVO slice


def kernel_body(tc, it, sh, xT, xq_bf, wqkv, wvt, wos, out):
    """Generator: yields after phase-1 emission (WVO partial + AllReduce +
    loads) and after phase-3 (scores); the code after the second yield
    emits the AllReduce-gated tail (VW = x@WVO, then p @ VW)."""
    nc = tc.nc
    sd = "left" if it % 2 == 0 else "right"
    strm, qtp, scp = sh["strm"], sh["qtp"], sh["scp"]
    lam_sb, bq_sb, ident_bf = sh["lam"], sh["bq"], sh["ident"]
    with (
        tc.tile_pool(name=f"carry{it}", bufs=1, side=sd) as carry,
        tc.tile_pool(name=f"dram{it}", bufs=1, space="DRAM", side=sd) as dram,
    ):
        ptile = carry.tile([P, NKC, SQ], BF16)   # p^T, [k_in, kc, q]
        r1s = [carry.tile([P, 1], F32, name=f"r1_{q}") for q in range(QB)]

        cc_in = dram.tile([D, DM], F32)
        cc_out = dram.tile([D, DM], F32)

        qkvT_q = qtp.tile([P, 4, SQ], F32R, tag="qq", bufs=1,
                          name=f"qq_{it}")
        qkvT_k = qtp.tile([P, 4, S], F32R, tag="qk", bufs=1, name=f"qk_{it}")

        # ---------- phase 1: WVO v-slice partial -> AllReduce --------------
        with tc.tile_pool(name=f"wps{it}", bufs=1, space="PSUM",
                          side=sd) as wps:
            wvt_sb = strm.tile([P, VC8, D], BF16, tag="wvt", bufs=1,
                               name=f"wvt_{it}")
            wos_sb = strm.tile([P, VC8, DM], BF16, tag="wos", bufs=1,
                               name=f"wos_{it}")
            xq_sb = strm.tile([P, KD, SQ], BF16, tag="xq", bufs=1,
                              name=f"xq_{it}")
            nc.sync.dma_start(wvt_sb, wvt)
            nc.sync.dma_start(wos_sb, wos)
            nc.sync.dma_start(
                xq_sb, xq_bf.rearrange("(dc p) s -> p dc s", p=P))

            # WVO_partial[d, m] = sum_{v in slice} WvT[v, d] * Wo[v, m]
            wvo_ps = [wps.tile([P, DM], F32, name=f"wvo_ps_{dc}")
                      for dc in range(KD)]
            for vc8 in range(VC8):
                for dc in range(KD):
                    nc.tensor.matmul(
                        wvo_ps[dc], wvt_sb[:, vc8, ts(dc, P)],
                        wos_sb[:, vc8, :],
                        start=(vc8 == 0), stop=(vc8 == VC8 - 1))
            wvo_stage = strm.tile([P, KD, DM], F32, tag="wvst", bufs=1,
                                  name=f"wvst_{it}")
            for dc in range(KD):
                nc.vector.tensor_copy(wvo_stage[:, dc], wvo_ps[dc])
            nc.scalar.dma_start(
                cc_in.rearrange("(dc p) m -> p dc m", p=P), wvo_stage)
            nc.gpsimd.collective_compute(
                "AllReduce", mybir.AluOpType.add,
                ins=[cc_in[:]], outs=[cc_out[:]],
                replica_groups=[[0, 1, 2, 3, 4, 5, 6, 7]],
            )
            # AllReduce-gated gather-in load on the (otherwise idle) gpsimd
            # queue, casting f32 -> bf16 in the DMA (SWDGE can cast)
            wvo_sb = strm.tile([P, KD, DM], BF16, tag="wvo", bufs=2,
                               name=f"wvo_{it}")
            nc.gpsimd.dma_start(
                wvo_sb, cc_out.rearrange("(dc p) m -> p dc m", p=P))

            # qkv-phase loads, streamed behind the tiny phase-1 loads
            xTs = strm.tile([P, KD, S], BF16, tag="xts", bufs=2,
                            name=f"xts_{it}")
            wq_sb = strm.tile([P, KD, 2 * D], BF16, tag="wq", bufs=1,
                              name=f"wq_{it}")
            nc.sync.dma_start(
                wq_sb, wqkv.rearrange("(dc p) m -> p dc m", p=P))
            for dc in range(KD):
                nc.sync.dma_start(xTs[:, dc], xT[ds(dc * P, P), :])

            # ---- pipeline point: previous body's tail goes here ----
            yield

        # ---------- phase 2: qkv projection ---------------------------------
        with tc.tile_pool(name=f"qps{it}", bufs=4,
                          space="PSUM", side=sd) as qps:
            for m in range(4):
                pt = qps.tile([P, SQ], F32, tag="ps")
                for dc in range(KD):
                    nc.tensor.matmul(
                        pt, wq_sb[:, dc, ts(m, P)], xq_sb[:, dc],
                        start=(dc == 0), stop=(dc == KD - 1))
                nc.scalar.activation(qkvT_q[:, m], pt, IDENT,
                                     bias=bq_sb[:, m : m + 1])
            for sn in range(SN):
                for m in range(4, MQ):
                    pt = qps.tile([P, 512], F32, tag="ps")
                    for dc in range(KD):
                        nc.tensor.matmul(
                            pt, wq_sb[:, dc, ts(m, P)],
                            xTs[:, dc, ts(sn, 512)],
                            start=(dc == 0), stop=(dc == KD - 1))
                    if (sn * 4 + m) % 2 == 0:
                        nc.scalar.activation(
                            qkvT_k[:, m - 4, ts(sn, 512)], pt, IDENT,
                            bias=bq_sb[:, m : m + 1])
                    else:
                        nc.vector.tensor_scalar_add(
                            qkvT_k[:, m - 4, ts(sn, 512)], pt,
                            bq_sb[:, m : m + 1])

        # ---------- phase 3: scores/softmax/combine/transpose --------------
        with tc.tile_pool(name=f"sps{it}", bufs=2, space="PSUM",
                          side=sd) as sps:
            pend2 = []

            def emit_scores(qb):
                ets = []
                sums = []
                for mi in range(2):
                    et = scp.tile([P, S], BF16, tag=f"e{mi}", bufs=2,
                                  name=f"e{mi}_{it}_{qb}")
                    st = scp.tile([P, 2], F32, tag=f"sum{mi}", bufs=3,
                                  name=f"sum{mi}_{it}_{qb}")
                    for half in range(2):
                        pt = sps.tile([P, 2, 512], F32, tag="ps",
                                      name=f"ps_{qb}_{mi}_{half}")
                        for knj in range(2):
                            kn = half * 2 + knj
                            for dc in range(2):
                                nc.tensor.matmul(
                                    pt[:, knj],
                                    qkvT_q[:, 2 * mi + dc, ts(qb, P)],
                                    qkvT_k[:, 2 * mi + dc, ts(kn, 512)],
                                    start=(dc == 0), stop=(dc == 1))
                        nc.scalar.activation(
                            et[:, ts(half, 1024)],
                            pt.rearrange("p a b -> p (a b)"), EXP,
                            scale=SCALE,
                            accum_out=st[:, half : half + 1])
                    ets.append(et)
                    sums.append(st)
                s1 = scp.tile([P, 1], F32, tag="s1", bufs=3,
                              name=f"s1_{it}_{qb}")
                nc.vector.reduce_sum(s1, sums[0], axis=AXX)
                nc.vector.reciprocal(r1s[qb], s1)
                s2 = scp.tile([P, 1], F32, tag="s2", bufs=3,
                              name=f"s2_{it}_{qb}")
                nc.vector.reduce_sum(s2, sums[1], axis=AXX)
                r2 = scp.tile([P, 1], F32, tag="r2", bufs=3,
                              name=f"r2_{it}_{qb}")
                nc.vector.reciprocal(r2, s2)
                u = scp.tile([P, 1], F32, tag="u", bufs=3,
                             name=f"u_{it}_{qb}")
                nc.vector.tensor_mul(u, s1, lam_sb)     # u = -lam*s1
                r2q = scp.tile([P, 1], F32, tag="r2q", bufs=3,
                               name=f"r2q_{it}_{qb}")
                nc.vector.tensor_mul(r2q, u, r2)        # r2q = -lam*s1/s2
                pend2.append((qb, ets, r2q))

            def emit_combine():
                qb, ets, r2q = pend2.pop(0)
                pb = scp.tile([P, S], BF16, tag="pb", bufs=2,
                              name=f"pb_{it}_{qb}")
                nc.vector.affine_then_add(pb, ets[1], ets[0], r2q, 0.0)
                for kc4 in range(NKC // 4):
                    tp = sps.tile([P, 4, P], BF16, tag="tp",
                                  name=f"tp_{qb}_{kc4}")
                    for j in range(4):
                        kc = kc4 * 4 + j
                        nc.tensor.matmul(tp[:, j], pb[:, ts(kc, P)],
                                         ident_bf, is_transpose=True)
                    nc.vector.tensor_copy(
                        ptile[:, ts(kc4, 4), ts(qb, P)], tp)

            for qb in range(QB):
                emit_scores(qb)
                if qb > 0:
                    emit_combine()
            emit_combine()

        # ---- pipeline point: tail emitted on next resume ------------------
        yield

        # ---------- tail: VW = x @ WVO, then p @ VW (AllReduce-gated) ------
        with tc.tile_pool(name=f"fps{it}", bufs=2, space="PSUM",
                          side=sd) as fps:
            vw_sb = scp.tile([P, NKC, DM], BF16, tag="vws", bufs=1,
                             name=f"vws_{it}")
            for kc in range(NKC):
                vw2 = fps.tile([P, DM], F32, tag="vw2", name=f"vw2_{kc}")
                for dc in range(KD):
                    nc.tensor.matmul(
                        vw2, xTs[:, dc, ts(kc, P)], wvo_sb[:, dc, :],
                        start=(dc == 0), stop=(dc == KD - 1))
                if kc % 2 == 0:
                    nc.scalar.activation(vw_sb[:, kc], vw2, IDENT)
                else:
                    nc.vector.tensor_copy(vw_sb[:, kc], vw2)
            for qb in range(QB):
                ft = fps.tile([P, DM], F32, tag="f", name=f"f_{qb}")
                for kc in range(NKC):
                    nc.tensor.matmul(
                        ft, ptile[:, kc, ts(qb, P)], vw_sb[:, kc, :],
                        start=(kc == 0), stop=(kc == NKC - 1))
                ofsb = scp.tile([P, DM], F32, tag="of", bufs=2,
                                name=f"of_{it}_{qb}")
                nc.scalar.activation(ofsb, ft, IDENT, scale=r1s[qb])
                nc.scalar.dma_start(out[ds(qb * P, P), :], ofsb)


def build_module(n_iters=1):
    nc = bacc.Bacc("TRN2", target_bir_lowering=False, debug=False)
    xT = nc.dram_tensor("xT", (D, S), BF16, kind="ExternalInput").ap()
    xq_bf = nc.dram_tensor("xq_bf", (D, SQ), BF16, kind="ExternalInput").ap()
    wqkv = nc.dram_tensor("wqkv", (D, 2 * D), BF16, kind="ExternalInput").ap()
    wvt = nc.dram_tensor("wvt", (P, VC8 * D), BF16, kind="ExternalInput").ap()
    wos = nc.dram_tensor("wos", (P, VC8 * DM), BF16,
                         kind="ExternalInput").ap()
    lamn = nc.dram_tensor("lamn", (P, 1), F32, kind="ExternalInput").ap()
    bq = nc.dram_tensor("bq", (P, MQ), F32, kind="ExternalInput").ap()
    out = nc.dram_tensor("out", (SQ, DM), F32, kind="ExternalOutput").ap()
    with tile.TileContext(nc) as tc:
        with ExitStack() as st:
            lite = st.enter_context(tc.tile_pool(name="lite", bufs=1))
            strm = st.enter_context(tc.tile_pool(name="strm", bufs=1))
            qtp = st.enter_context(tc.tile_pool(name="qtp", bufs=1))
            scp = st.enter_context(tc.tile_pool(name="scp", bufs=1))
            lam_sb = lite.tile([P, 1], F32)
            bq_sb = lite.tile([P, MQ], F32)
            ident_f32 = lite.tile([P, P], F32)
            ident_bf = lite.tile([P, P], BF16)
            nc.scalar.dma_start(lam_sb, lamn)
            nc.scalar.dma_start(bq_sb, bq)
            make_identity(nc, ident_f32)
            nc.vector.tensor_copy(ident_bf, ident_f32)
            sh = {"strm": strm, "qtp": qtp, "scp": scp,
                  "lam": lam_sb, "bq": bq_sb, "ident": ident_bf}
            prev = None
            for it in range(n_iters):
                g = kernel_body(tc, it, sh, xT, xq_bf, wqkv, wvt, wos, out)
                next(g)               # phase 1 of body `it`
                if prev is not None:  # tail of body `it-1` lands here
                    try:
                        next(prev)
                    except StopIteration:
                        pass
                for _ in g:           # phases 2-3, stop at the pre-tail yield
                    break
                prev = g
            try:
                next(prev)            # final body's tail
            except StopIteration:
                pass
    nc.compile()
    return nc


_NC = None


def _get_module():
    global _NC
    if _NC is None:
        _NC = build_module()
    return _NC


def host_prep(**inputs):
    """Host-side input prep: returns (in_maps, lam, host_bias)."""
    x = np.asarray(inputs["x"], np.float32)
    Wqkv = np.asarray(inputs["Wqkv"], np.float32)
    bqkv = np.asarray(inputs["bqkv"], np.float32)
    Wv = np.asarray(inputs["Wv"], np.float32)
    bv = np.asarray(inputs["bv"], np.float32)
    Wo = np.asarray(inputs["Wo"], np.float32)
    bo = np.asarray(inputs["bo"], np.float32)
    lq1 = np.asarray(inputs["lq1"], np.float32)
    lk1 = np.asarray(inputs["lk1"], np.float32)
    lq2 = np.asarray(inputs["lq2"], np.float32)
    lk2 = np.asarray(inputs["lk2"], np.float32)

    lam = float(
        np.exp(np.sum(lq1 * lk1, dtype=np.float32))
        - np.exp(np.sum(lq2 * lk2, dtype=np.float32))
        + (LAMBDA_INIT - 0.6 * math.exp(-0.3 * LAYER_INDEX))
    )
    bq_host = np.ascontiguousarray(bqkv.reshape(MQ, P).T)
    lam_host = np.full((P, 1), -lam, np.float32)

    wq_bf = Wqkv.astype(ml_dtypes.bfloat16)
    # per-core v-slices of Wv^T and Wo, [(vc p), d/m] -> [p, (vc d/m)]
    WvT = np.ascontiguousarray(Wv.T)       # [8192, 512]
    wvt_r = WvT.reshape(VDIM // P, P, D).transpose(1, 0, 2)   # [128, 64, 512]
    wo_r = Wo.reshape(VDIM // P, P, DM).transpose(1, 0, 2)    # [128, 64, 512]

    in_maps = []
    for c in range(8):
        b, g = divmod(c, G)
        xTb = np.ascontiguousarray(x[b].T).astype(ml_dtypes.bfloat16)
        vlo, vhi = c * VC8, (c + 1) * VC8
        in_maps.append({
            "xT": xTb,
            "xq_bf": np.ascontiguousarray(xTb[:, g * SQ : (g + 1) * SQ]),
            "wqkv": wq_bf,
            "wvt": np.ascontiguousarray(
                wvt_r[:, vlo:vhi].reshape(P, -1)).astype(ml_dtypes.bfloat16),
            "wos": np.ascontiguousarray(
                wo_r[:, vlo:vhi].reshape(P, -1)).astype(ml_dtypes.bfloat16),
            "lamn": lam_host,
            "bq": bq_host,
        })
    # sum_k diff_attn[q, :] == 1 - lam exactly, so bv and bo fold into a
    # constant per-output-column correction.
    host_bias = ((1.0 - lam) * bv) @ Wo + bo
    return in_maps, lam, host_bias.astype(np.float32)


def kernel(**inputs):
    in_maps, _lam, host_bias = host_prep(**inputs)
    nc = _get_module()
    res = None
    for attempt in range(3):
        try:
            res = bass_utils.run_bass_kernel_spmd(
                nc, in_maps, core_ids=list(range(8)))
            break
        except Exception:
            # transient NRT_EXEC_UNIT_UNRECOVERABLE flakes have been seen on
            # the first execution of a freshly compiled NEFF; retry
            if attempt == 2:
                raise
            import time
            time.sleep(2.0)
    out = np.empty((B, S, DM), np.float32)
    for c in range(8):
        b, g = divmod(c, G)
        out[b, g * SQ : (g + 1) * SQ, :] = res.results[c]["out"]
    out += host_bias
    return out
